# revision 24
# baseline (speedup 1.0000x reference)
"""Bidirectional Mamba TRN2 kernel (8 NeuronCores, SPMD).

Sharding: core c owns batch b = c//4 and 3 dtiles of 128 d_inner channels
(both scan directions). x_proj partials are AllReduced per direction
(groups {0..3}, {4..7}); out_proj partials are summed on the host.

Engine plan (per core):
  PE      : in_proj / conv (diag shifted matmuls) / x_proj / dt_proj,
            identity-matmul PSUM accumulation of sum_n C_n*h_n, out_proj.
  ScalarE : all PSUM evacs, sigmoid/softplus pieces, the 16 dA=exp(A_n*delta).
  DVE     : native tensor_tensor_scan (full-speed forward only; the backward
            branch is time-reversed at evac points so its scans also run
            forward), hc=h*C muls, du/gate glue.
  GPSIMD  : d1=du*B muls (tensor_tensor), collectives.
  DMA     : B/C row partition-broadcasts, spills (u/sz/m/y), I/O.
"""
import numpy as np
import ml_dtypes
from contextlib import ExitStack

import concourse.bass as bass
import concourse.bacc as bacc
import concourse.tile as tile
from concourse import mybir, library_config
from concourse.bass_utils import run_bass_kernel_spmd

B, L, D = 2, 4096, 768
DI, DS, DTR, KC = 1536, 16, 48, 4
NCORES = 8
NPAIR = 3                 # dtiles per core
P = 128
NKT = D // P              # 6 K-tiles for in_proj
LC = 512                  # PSUM chunk
NCH = L // LC             # 8
LH = 2048                 # scan half length
E = DTR + 2 * DS          # 80

f32 = mybir.dt.float32
bf16 = mybir.dt.bfloat16
ALU = mybir.AluOpType
AF = mybir.ActivationFunctionType

bfdt = ml_dtypes.bfloat16

# When True, avoid activation functions the CoreSim interpreter lacks
# (Silu/Softplus) and use Sigmoid/Exp+Ln compositions instead. Hardware
# builds use the native single-pass functions.
SIM_COMPAT = False


def build_module():
    sim_compat = SIM_COMPAT
    nc = bacc.Bacc("TRN2", target_bir_lowering=False, debug=False,
                   num_devices=NCORES)

    # ---- external inputs (per-core data; same tensor names on all cores) ----
    hT16 = nc.dram_tensor("hT16", [D, L], bf16, kind="ExternalInput")
    w_in16 = nc.dram_tensor("w_in16", [D, 2 * NPAIR * P], bf16, kind="ExternalInput")
    convd16 = nc.dram_tensor("convd16", [2, NPAIR, KC, P, P], bf16, kind="ExternalInput")
    cbias = nc.dram_tensor("cbias", [2, NPAIR, P], f32, kind="ExternalInput")
    w_xp16 = nc.dram_tensor("w_xp16", [2, NPAIR, P, E], bf16, kind="ExternalInput")
    w_dt16 = nc.dram_tensor("w_dt16", [2, NPAIR, DTR, P], bf16, kind="ExternalInput")
    dtb = nc.dram_tensor("dtb", [2, NPAIR, P], f32, kind="ExternalInput")
    Acol = nc.dram_tensor("Acol", [2, NPAIR, P, DS], f32, kind="ExternalInput")
    Dvec = nc.dram_tensor("Dvec", [2, NPAIR, P], f32, kind="ExternalInput")
    w_out16 = nc.dram_tensor("w_out16", [NPAIR, P, D], bf16, kind="ExternalInput")
    ident_in = nc.dram_tensor("ident_in", [P, P], bf16, kind="ExternalInput")
    out_part = nc.dram_tensor("out_part", [D, L], f32, kind="ExternalOutput")

    # ---- internal DRAM ----
    cc_in = nc.dram_tensor("cc_in", [2, E, L], f32)
    cc_out = nc.dram_tensor("cc_out", [2, E, L], f32)
    bc16d = nc.dram_tensor("bc16d", [2, 2 * DS, L], bf16)
    u_dram = nc.dram_tensor("u_dram", [2, NPAIR, P, L], bf16)
    szd = nc.dram_tensor("szd", [NPAIR, P, L], bf16)
    m_dram = nc.dram_tensor("m_dram", [NPAIR, P, L], bf16)
    y_dram = nc.dram_tensor("y_dram", [NPAIR, P, L], bf16)

    with tile.TileContext(nc) as tc, ExitStack() as top:
        wp = top.enter_context(tc.tile_pool(name="wp", bufs=1))

        nc.gpsimd.load_library(library_config.proxy)

        # ---- persistent small weights ----
        cb_sb = wp.tile([P, 2, NPAIR], f32, tag="cb", name="cb")
        nc.sync.dma_start(cb_sb[:], cbias.ap().rearrange("d j p -> p d j"))
        dtb_sb = wp.tile([P, 2, NPAIR], f32, tag="dtb", name="dtb")
        nc.sync.dma_start(dtb_sb[:], dtb.ap().rearrange("d j p -> p d j"))
        Acol_sb = wp.tile([P, 2, NPAIR, DS], f32, tag="Acol", name="Acol")
        nc.sync.dma_start(Acol_sb[:], Acol.ap().rearrange("d j p n -> p d j n"))
        D_sb = wp.tile([P, 2, NPAIR], f32, tag="Dsb", name="Dsb")
        nc.sync.dma_start(D_sb[:], Dvec.ap().rearrange("d j p -> p d j"))
        w_dt_sb = wp.tile([DTR, 2, NPAIR, P], bf16, tag="w_dt", name="w_dt")
        nc.sync.dma_start(w_dt_sb[:], w_dt16.ap().rearrange("d j r m -> r d j m"))
        w_out_sb = wp.tile([P, NPAIR, D], bf16, tag="w_out", name="w_out")
        nc.sync.dma_start(w_out_sb[:], w_out16.ap().rearrange("j p c -> p j c"))
        ident_sb = wp.tile([P, P], bf16, tag="ident", name="ident")
        nc.sync.dma_start(ident_sb[:], ident_in.ap())

        # ================= Phase 1: in_proj, silu(z), conv, u, dbl ==========
        with ExitStack() as p1:
            p1w = p1.enter_context(tc.tile_pool(name="p1w", bufs=1))
            x16p = p1.enter_context(tc.tile_pool(name="x16p", bufs=1))
            tp1 = p1.enter_context(tc.tile_pool(name="tp1", bufs=2))

            hT_sb = p1w.tile([P, NKT, L], bf16, tag="hT", name="hT")
            nc.sync.dma_start(hT_sb[:], hT16.ap().rearrange("(k p) l -> p k l", p=P))
            w_in_sb = p1w.tile([P, NKT, 2 * NPAIR * P], bf16, tag="w_in", name="w_in")
            nc.sync.dma_start(w_in_sb[:],
                              w_in16.ap().rearrange("(k p) c -> p k c", p=P))
            convd_sb = p1w.tile([P, 2, NPAIR, KC, P], bf16, tag="convd", name="convd")
            nc.sync.dma_start(convd_sb[:],
                              convd16.ap().rearrange("d j k p m -> p d j k m"))
            w_xp_sb = p1w.tile([P, 2, NPAIR, E], bf16, tag="w_xp", name="w_xp")
            nc.sync.dma_start(w_xp_sb[:],
                              w_xp16.ap().rearrange("d j p e -> p d j e"))

            x16 = [x16p.tile([P, L], bf16, tag=f"x16_{j}", name=f"x16_{j}")
                   for j in range(NPAIR)]

            # ---- in_proj (+ silu(z) -> szd) ----
            def in_proj_half(j, s, sz16=None):
                for c in range(NCH):
                    cols = slice(c * LC, (c + 1) * LC)
                    ps = xzps.tile([P, LC], f32, tag="xzps", name="xzps")
                    wcol = (j * 2 + s) * P
                    for kt in range(NKT):
                        nc.tensor.matmul(
                            ps[:], w_in_sb[:, kt, wcol:wcol + P],
                            hT_sb[:, kt, cols],
                            start=(kt == 0), stop=(kt == NKT - 1))
                    if s == 0:
                        nc.scalar.copy(x16[j][:, cols], ps[:])
                    elif sim_compat:
                        z16 = tp1.tile([P, LC], bf16, tag="z16", name="z16")
                        nc.scalar.copy(z16[:], ps[:])
                        sg = tp1.tile([P, LC], bf16, tag="zsg", name="zsg")
                        nc.scalar.activation(sg[:], ps[:], AF.Sigmoid)
                        nc.vector.tensor_tensor(
                            sz16[:, cols], z16[:], sg[:], op=ALU.mult)
                    else:
                        nc.scalar.activation(sz16[:, cols], ps[:], AF.Silu)

            xzps = p1.enter_context(
                tc.tile_pool(name="xzps", bufs=2, space="PSUM"))
            # x halves first so conv-a/dbl-a/AllReduce-a launch early;
            # z halves later, overlapped with the collective.
            for j in range(NPAIR):
                in_proj_half(j, 0)

            # ---- conv + silu -> u (dir b time-reversed), dbl, collective ----
            for dr in range(2):
                with ExitStack() as s2:
                    up = s2.enter_context(tc.tile_pool(name=f"up{dr}", bufs=1))
                    cps = s2.enter_context(
                        tc.tile_pool(name=f"cps{dr}", bufs=2, space="PSUM"))
                    dblps = s2.enter_context(
                        tc.tile_pool(name=f"dblps{dr}", bufs=2, space="PSUM"))
                    u16 = [up.tile([P, L], bf16, tag=f"u16_{j}", name=f"u16_{dr}{j}")
                           for j in range(NPAIR)]
                    for j in range(NPAIR):
                        urev = u16[j][:, ::-1] if dr == 1 else None
                        for c in range(NCH):
                            c0 = c * LC
                            ps = cps.tile([P, LC], f32, tag="cps", name="cps")
                            if dr == 0:
                                # tap k reads x[t-3+k]; k=3 full width first
                                nc.tensor.matmul(
                                    ps[:], convd_sb[:, dr, j, 3, :],
                                    x16[j][:, c0:c0 + LC],
                                    start=True, stop=False)
                                for k in range(3):
                                    sh = 3 - k
                                    if c == 0:
                                        nc.tensor.matmul(
                                            ps[:, sh:LC],
                                            convd_sb[:, dr, j, k, :],
                                            x16[j][:, 0:LC - sh],
                                            start=False, stop=(k == 2))
                                    else:
                                        nc.tensor.matmul(
                                            ps[:], convd_sb[:, dr, j, k, :],
                                            x16[j][:, c0 - sh:c0 + LC - sh],
                                            start=False, stop=(k == 2))
                            else:
                                # flipped tap jj reads x[t+jj]; jj=0 full first
                                nc.tensor.matmul(
                                    ps[:], convd_sb[:, dr, j, 0, :],
                                    x16[j][:, c0:c0 + LC],
                                    start=True, stop=False)
                                for jj in range(1, KC):
                                    if c == NCH - 1:
                                        nc.tensor.matmul(
                                            ps[:, 0:LC - jj],
                                            convd_sb[:, dr, j, jj, :],
                                            x16[j][:, c0 + jj:L],
                                            start=False, stop=(jj == KC - 1))
                                    else:
                                        nc.tensor.matmul(
                                            ps[:], convd_sb[:, dr, j, jj, :],
                                            x16[j][:, c0 + jj:c0 + LC + jj],
                                            start=False, stop=(jj == KC - 1))
                            if dr == 0:
                                uout = u16[j][:, c0:c0 + LC]
                            else:
                                uout = urev[:, c0:c0 + LC]
                            if sim_compat:
                                pre = tp1.tile([P, LC], bf16, tag="upre", name="upre")
                                nc.scalar.activation(pre[:], ps[:], AF.Identity,
                                                     bias=cb_sb[:, dr, j:j + 1])
                                sgu = tp1.tile([P, LC], bf16, tag="usg", name="usg")
                                nc.scalar.activation(sgu[:], ps[:], AF.Sigmoid,
                                                     bias=cb_sb[:, dr, j:j + 1])
                                nc.vector.tensor_tensor(uout, pre[:], sgu[:],
                                                        op=ALU.mult)
                            else:
                                nc.scalar.activation(uout, ps[:], AF.Silu,
                                                     bias=cb_sb[:, dr, j:j + 1])
                        nc.sync.dma_start(u_dram.ap()[dr, j], u16[j][:])

                    # dbl partials over this core's channels (tau coords for b)
                    dbl_sb = up.tile([E, L], f32, tag="dbl", name=f"dbl{dr}")
                    for c in range(NCH):
                        cols = slice(c * LC, (c + 1) * LC)
                        dps = dblps.tile([E, LC], f32, tag="dblps", name="dblps")
                        for j in range(NPAIR):
                            # u16 is tau-ordered for dr=1; dbl follows branch coords
                            nc.tensor.matmul(
                                dps[:], w_xp_sb[:, dr, j, :], u16[j][:, cols],
                                start=(j == 0), stop=(j == NPAIR - 1))
                        nc.scalar.copy(dbl_sb[:, cols], dps[:])
                    nc.sync.dma_start(cc_in.ap()[dr], dbl_sb[:])

                nc.gpsimd.collective_compute(
                    "AllReduce", ALU.add,
                    replica_groups=[[0, 1, 2, 3], [4, 5, 6, 7]],
                    ins=[cc_in.ap()[dr]], outs=[cc_out.ap()[dr]])

                if dr == 0:
                    # z halves + silu gate, overlapped with AllReduce-a
                    for j in range(NPAIR):
                        sz16 = tp1.tile([P, L], bf16, tag="sz16", name="sz16")
                        in_proj_half(j, 1, sz16)
                        nc.sync.dma_start(szd.ap()[j], sz16[:])

        # ================= Phase 2: delta, scans, gate =======================
        for dr in range(2):
            with ExitStack() as p2:
                dlp = p2.enter_context(tc.tile_pool(name="dlp", bufs=1))
                bp = p2.enter_context(tc.tile_pool(name="bp", bufs=2))
                trp = p2.enter_context(tc.tile_pool(name="trp", bufs=2))
                hp = p2.enter_context(tc.tile_pool(name="hp", bufs=2))
                dup = p2.enter_context(tc.tile_pool(name="dup", bufs=2))
                dup1 = p2.enter_context(tc.tile_pool(name="dup1", bufs=1))
                gp = p2.enter_context(tc.tile_pool(name="gp", bufs=1))

                # dt_low + B/C rows, bf16-cast via gpsimd DMA.
                # dtlow borrows the gate pool's y16 tag (disjoint lifetime).
                dtlow = gp.tile([DTR, L], bf16, tag="y16", name="dtlow")
                nc.gpsimd.dma_start(dtlow[:], cc_out.ap()[dr, 0:DTR, :])
                bc16 = dlp.tile([2 * DS, L], bf16, tag="bc16", name="bc16")
                nc.gpsimd.dma_start(bc16[:], cc_out.ap()[dr, DTR:E, :])
                nc.sync.dma_start(bc16d.ap()[dr], bc16[:])

                # delta per j (kept for the whole dir)
                d16 = [dlp.tile([P, L], bf16, tag=f"d16_{j}", name=f"d16_{dr}{j}")
                       for j in range(NPAIR)]
                with ExitStack() as sdt:
                    psd = sdt.enter_context(
                        tc.tile_pool(name="psd", bufs=2, space="PSUM"))
                    # e16 borrows gate-pool tags (disjoint lifetimes, same shape)
                    e16 = [gp.tile([P, L], bf16, tag=t, name=f"e16{t}")
                           for t in ("u_a", "m_a", "sz")]
                    for j in range(NPAIR):
                        for c in range(NCH):
                            cols = slice(c * LC, (c + 1) * LC)
                            dps = psd.tile([P, LC], f32, tag="dtps", name="dtps")
                            nc.tensor.matmul(dps[:], w_dt_sb[:, dr, j, :],
                                             dtlow[:, cols],
                                             start=True, stop=True)
                            nc.scalar.activation(e16[j][:, cols], dps[:],
                                                 AF.Exp,
                                                 bias=dtb_sb[:, dr, j:j + 1])
                    for j in range(NPAIR):
                        for c in range(NCH):
                            cols = slice(c * LC, (c + 1) * LC)
                            nc.scalar.activation(d16[j][:, cols],
                                                 e16[j][:, cols], AF.Ln,
                                                 bias=1.0)

                mps = p2.enter_context(
                    tc.tile_pool(name="mps", bufs=1, space="PSUM"))
                for j in range(NPAIR):
                    u16j = dup.tile([P, L], bf16, tag="u16j", name="u16j")
                    nc.sync.dma_start(u16j[:], u_dram.ap()[dr, j])
                    du16 = dup1.tile([P, L], bf16, tag="du16", name="du16")
                    for hf in range(2):
                        hc_ = slice(hf * LH, (hf + 1) * LH)
                        nc.vector.tensor_tensor(du16[:, hc_], d16[j][:, hc_],
                                                u16j[:, hc_], op=ALU.mult)

                    m_ps = [mps.tile([P, LC], f32, tag=f"mps{c}", name=f"mps{c}")
                            for c in range(NCH)]
                    for n in range(DS):
                        Brep = bp.tile([P, L], bf16, tag="Brep", name="Brep")
                        nc.sync.dma_start(
                            Brep[:],
                            bc16d.ap()[dr, n:n + 1, :].partition_broadcast(P))
                        Crep = bp.tile([P, L], bf16, tag="Crep", name="Crep")
                        nc.sync.dma_start(
                            Crep[:],
                            bc16d.ap()[dr, DS + n:DS + n + 1, :].partition_broadcast(P))
                        d1 = trp.tile([P, L], bf16, tag="d1", name="d1")
                        nc.gpsimd.tensor_tensor(d1[:], du16[:], Brep[:],
                                                op=ALU.mult)
                        dA = trp.tile([P, L], bf16, tag="dA", name="dA")
                        nc.scalar.activation(dA[:], d16[j][:], AF.Exp,
                                             scale=Acol_sb[:, dr, j, n:n + 1])
                        h = hp.tile([P, L], bf16, tag="h", name="h")
                        nc.vector.tensor_tensor_scan(
                            h[:], dA[:], d1[:], 0.0,
                            op0=ALU.mult, op1=ALU.add)
                        hcm = trp.tile([P, L], bf16, tag="hcm", name="hcm")
                        for hf in range(2):
                            hc_ = slice(hf * LH, (hf + 1) * LH)
                            nc.vector.tensor_tensor(hcm[:, hc_], h[:, hc_],
                                                    Crep[:, hc_], op=ALU.mult)
                        for c4 in range(NCH):
                            cc = slice(c4 * LC, (c4 + 1) * LC)
                            nc.tensor.matmul(
                                m_ps[c4][:], ident_sb[:], hcm[:, cc],
                                start=(n == 0), stop=(n == DS - 1))

                    if dr == 0:
                        m16 = gp.tile([P, L], bf16, tag="m16", name="m16")
                        for c in range(NCH):
                            cols = slice(c * LC, (c + 1) * LC)
                            nc.scalar.copy(m16[:, cols], m_ps[c][:])
                        nc.sync.dma_start(m_dram.ap()[j], m16[:])
                    else:
                        # combine: y = (m_a + m_b_rev + Da*u_a + Db*u_b_rev)*sz
                        u_a = gp.tile([P, L], bf16, tag="u_a", name="u_a")
                        nc.sync.dma_start(u_a[:], u_dram.ap()[0, j])
                        m_a = gp.tile([P, L], bf16, tag="m_a", name="m_a")
                        nc.sync.dma_start(m_a[:], m_dram.ap()[j])
                        sz = gp.tile([P, L], bf16, tag="sz", name="sz")
                        nc.sync.dma_start(sz[:], szd.ap()[j])
                        y16 = gp.tile([P, L], bf16, tag="y16", name="y16")
                        ubrev = u16j[:, ::-1]
                        for c in range(NCH):
                            cols = slice(c * LC, (c + 1) * LC)
                            q1 = gp.tile([P, LC], f32, tag="q1", name="q1")
                            q2 = gp.tile([P, LC], f32, tag="q2", name="q2")
                            nc.vector.scalar_tensor_tensor(
                                q1[:], u_a[:, cols], D_sb[:, 0, j:j + 1],
                                m_ps[NCH - 1 - c][:, ::-1],
                                op0=ALU.mult, op1=ALU.add)
                            nc.vector.scalar_tensor_tensor(
                                q2[:], ubrev[:, cols], D_sb[:, 1, j:j + 1],
                                m_a[:, cols], op0=ALU.mult, op1=ALU.add)
                            s32 = gp.tile([P, LC], f32, tag="s32", name="s32")
                            nc.vector.tensor_tensor(s32[:], q1[:], q2[:],
                                                    op=ALU.add)
                            nc.vector.tensor_tensor(y16[:, cols], s32[:],
                                                    sz[:, cols], op=ALU.mult)
                        nc.sync.dma_start(y_dram.ap()[j], y16[:])

        # ================= out_proj ==========================================
        with ExitStack() as p3:
            yp = p3.enter_context(tc.tile_pool(name="yp", bufs=2))
            osp = p3.enter_context(tc.tile_pool(name="osp", bufs=2))
            ops_pool = p3.enter_context(
                tc.tile_pool(name="ops", bufs=2, space="PSUM"))
            for c in range(NCH):
                cols = slice(c * LC, (c + 1) * LC)
                ygc = []
                for j in range(NPAIR):
                    yg = yp.tile([P, LC], bf16, tag=f"yg{j}", name=f"yg{j}")
                    nc.sync.dma_start(yg[:], y_dram.ap()[j, :, cols])
                    ygc.append(yg)
                for ot in range(D // P):
                    ops_ = ops_pool.tile([P, LC], f32, tag="outps", name="outps")
                    for j in range(NPAIR):
                        nc.tensor.matmul(
                            ops_[:], w_out_sb[:, j, ot * P:(ot + 1) * P],
                            ygc[j][:], start=(j == 0), stop=(j == NPAIR - 1))
                    osb = osp.tile([P, LC], f32, tag="osb", name="osb")
                    nc.scalar.copy(osb[:], ops_[:])
                    nc.sync.dma_start(
                        out_part.ap()[ot * P:(ot + 1) * P, cols], osb[:])

    nc.compile()
    return nc


def _prep_core_inputs(inputs, core):
    """Host-side slicing/transposition of full inputs for one core."""
    b = core // 4
    dtiles = [(core % 4) * NPAIR + k for k in range(NPAIR)]
    chans = np.concatenate([np.arange(dt * P, (dt + 1) * P) for dt in dtiles])

    hid = np.asarray(inputs['hidden_states'])
    w_in_full = np.asarray(inputs['in_proj_w'])
    w_out_full = np.asarray(inputs['out_proj_w'])

    per_dir = {}
    for d, sfx in enumerate(('a', 'b')):
        per_dir[d] = dict(
            cw=np.asarray(inputs[f'conv_w_{sfx}'])[chans],
            cb=np.asarray(inputs[f'conv_b_{sfx}'])[chans],
            xp=np.asarray(inputs[f'x_proj_{sfx}_w'])[:, chans],
            dtp=np.asarray(inputs[f'dt_proj_{sfx}_w'])[chans],
            dtb=np.asarray(inputs[f'dt_bias_{sfx}'])[chans],
            A=-np.exp(np.asarray(inputs[f'A_{sfx}_log'])[chans]),
            Dv=np.asarray(inputs[f'D_{sfx}'])[chans],
        )

    w_in_cols = np.empty((D, 2 * NPAIR * P), np.float32)
    for j in range(NPAIR):
        ch_j = chans[j * P:(j + 1) * P]
        w_in_cols[:, (2 * j) * P:(2 * j + 1) * P] = w_in_full[ch_j].T
        w_in_cols[:, (2 * j + 1) * P:(2 * j + 2) * P] = w_in_full[DI + ch_j].T

    # conv taps as diagonal matrices (dir b taps host-flipped)
    convd = np.zeros((2, NPAIR, KC, P, P), np.float32)
    ar = np.arange(P)
    for d in range(2):
        cw = per_dir[d]['cw'].reshape(NPAIR, P, KC)
        if d == 1:
            cw = cw[:, :, ::-1]
        for j in range(NPAIR):
            for k in range(KC):
                convd[d, j, k, ar, ar] = cw[j, :, k]

    out = {
        'hT16': hid[b].T.astype(bfdt),
        'w_in16': w_in_cols.astype(bfdt),
        'convd16': convd.astype(bfdt),
        'cbias': np.ascontiguousarray(
            np.stack([per_dir[d]['cb'].reshape(NPAIR, P) for d in range(2)])
        ).astype(np.float32),
        'w_xp16': np.ascontiguousarray(
            np.stack([per_dir[d]['xp'].T.reshape(NPAIR, P, E)
                      for d in range(2)])).astype(bfdt),
        'w_dt16': np.ascontiguousarray(
            np.stack([per_dir[d]['dtp'].reshape(NPAIR, P, DTR)
                      .transpose(0, 2, 1) for d in range(2)])).astype(bfdt),
        'dtb': np.ascontiguousarray(
            np.stack([per_dir[d]['dtb'].reshape(NPAIR, P)
                      for d in range(2)])).astype(np.float32),
        'Acol': np.ascontiguousarray(
            np.stack([per_dir[d]['A'].reshape(NPAIR, P, DS)
                      for d in range(2)])).astype(np.float32),
        'Dvec': np.ascontiguousarray(
            np.stack([per_dir[d]['Dv'].reshape(NPAIR, P)
                      for d in range(2)])).astype(np.float32),
        'w_out16': np.ascontiguousarray(
            w_out_full[:, chans].T.reshape(NPAIR, P, D)).astype(bfdt),
        'ident_in': np.eye(P, dtype=bfdt),
    }
    return {k: np.ascontiguousarray(v) for k, v in out.items()}


_module_cache = {}


def _get_module():
    key = ('sim' if SIM_COMPAT else 'hw')
    if key not in _module_cache:
        _module_cache[key] = build_module()
    return _module_cache[key]


def kernel(**inputs):
    nc = _get_module()
    in_maps = [_prep_core_inputs(inputs, c) for c in range(NCORES)]
    res = run_bass_kernel_spmd(nc, in_maps, list(range(NCORES)))
    out = np.zeros((B, L, D), np.float32)
    for c in range(NCORES):
        out[c // 4] += res.results[c]['out_part'].T
    return out


# revision 28
# speedup vs baseline: 1.5654x; 1.5654x over previous
"""Bidirectional Mamba TRN2 kernel (8 NeuronCores, SPMD).

Sharding: core c owns batch b = c//4 and 3 dtiles of 128 d_inner channels
(both scan directions). x_proj partials are AllReduced per direction
(groups {0..3}, {4..7}); out_proj partials are summed on the host.

Engine plan (per core):
  PE      : in_proj / conv (diag shifted matmuls) / x_proj / dt_proj,
            identity-matmul PSUM accumulation of sum_n C_n*h_n, out_proj.
  ScalarE : all PSUM evacs, sigmoid/softplus pieces, the 16 dA=exp(A_n*delta).
  DVE     : native tensor_tensor_scan (full-speed forward only; the backward
            branch is time-reversed at evac points so its scans also run
            forward), hc=h*C muls, du/gate glue.
  GPSIMD  : d1=du*B muls (tensor_tensor), collectives.
  DMA     : B/C row partition-broadcasts, spills (u/sz/m/y), I/O.
"""
import numpy as np
import ml_dtypes
from contextlib import ExitStack

import concourse.bass as bass
import concourse.bacc as bacc
import concourse.tile as tile
from concourse import mybir, library_config
from concourse.bass_utils import run_bass_kernel_spmd

B, L, D = 2, 4096, 768
DI, DS, DTR, KC = 1536, 16, 48, 4
NCORES = 8
NPAIR = 3                 # dtiles per core
P = 128
NKT = D // P              # 6 K-tiles for in_proj
LC = 512                  # PSUM chunk
NCH = L // LC             # 8
LH = 2048                 # scan half length
E = DTR + 2 * DS          # 80

f32 = mybir.dt.float32
bf16 = mybir.dt.bfloat16
ALU = mybir.AluOpType
AF = mybir.ActivationFunctionType

bfdt = ml_dtypes.bfloat16

# When True, avoid activation functions the CoreSim interpreter lacks
# (Silu/Softplus) and use Sigmoid/Exp+Ln compositions instead. Hardware
# builds use the native single-pass functions.
SIM_COMPAT = False


def build_module():
    sim_compat = SIM_COMPAT
    nc = bacc.Bacc("TRN2", target_bir_lowering=False, debug=False,
                   num_devices=NCORES)

    # ---- external inputs (per-core data; same tensor names on all cores) ----
    hT16 = nc.dram_tensor("hT16", [D, L], bf16, kind="ExternalInput")
    w_in16 = nc.dram_tensor("w_in16", [D, 2 * NPAIR * P], bf16, kind="ExternalInput")
    convd16 = nc.dram_tensor("convd16", [2, NPAIR, KC, P, P], bf16, kind="ExternalInput")
    cbias = nc.dram_tensor("cbias", [2, NPAIR, P], f32, kind="ExternalInput")
    w_xp16 = nc.dram_tensor("w_xp16", [2, NPAIR, P, E], bf16, kind="ExternalInput")
    w_dt16 = nc.dram_tensor("w_dt16", [2, NPAIR, DTR, P], bf16, kind="ExternalInput")
    dtb = nc.dram_tensor("dtb", [2, NPAIR, P], f32, kind="ExternalInput")
    Acol = nc.dram_tensor("Acol", [2, NPAIR, P, DS], f32, kind="ExternalInput")
    Dvec = nc.dram_tensor("Dvec", [2, NPAIR, P], f32, kind="ExternalInput")
    w_out16 = nc.dram_tensor("w_out16", [NPAIR, P, D], bf16, kind="ExternalInput")
    ident_in = nc.dram_tensor("ident_in", [P, P], bf16, kind="ExternalInput")
    out_part = nc.dram_tensor("out_part", [D, L], f32, kind="ExternalOutput")

    # ---- internal DRAM ----
    cc_in = nc.dram_tensor("cc_in", [2, E, L], bf16)
    cc_out = nc.dram_tensor("cc_out", [2, E, L], bf16)
    u_dram = nc.dram_tensor("u_dram", [2, NPAIR, P, L], bf16)
    szd = nc.dram_tensor("szd", [NPAIR, P, L], bf16)
    m_dram = nc.dram_tensor("m_dram", [NPAIR, P, L], bf16)
    y_dram = nc.dram_tensor("y_dram", [NPAIR, P, L], bf16)

    with tile.TileContext(nc) as tc, ExitStack() as top:
        wp = top.enter_context(tc.tile_pool(name="wp", bufs=1))

        nc.gpsimd.load_library(library_config.proxy)

        # ---- persistent small weights ----
        cb_sb = wp.tile([P, 2, NPAIR], f32, tag="cb", name="cb")
        nc.sync.dma_start(cb_sb[:], cbias.ap().rearrange("d j p -> p d j"))
        dtb_sb = wp.tile([P, 2, NPAIR], f32, tag="dtb", name="dtb")
        nc.sync.dma_start(dtb_sb[:], dtb.ap().rearrange("d j p -> p d j"))
        Acol_sb = wp.tile([P, 2, NPAIR, DS], f32, tag="Acol", name="Acol")
        nc.sync.dma_start(Acol_sb[:], Acol.ap().rearrange("d j p n -> p d j n"))
        D_sb = wp.tile([P, 2, NPAIR], f32, tag="Dsb", name="Dsb")
        nc.sync.dma_start(D_sb[:], Dvec.ap().rearrange("d j p -> p d j"))
        w_dt_sb = wp.tile([DTR, 2, NPAIR, P], bf16, tag="w_dt", name="w_dt")
        nc.sync.dma_start(w_dt_sb[:], w_dt16.ap().rearrange("d j r m -> r d j m"))
        w_out_sb = wp.tile([P, NPAIR, D], bf16, tag="w_out", name="w_out")
        nc.sync.dma_start(w_out_sb[:], w_out16.ap().rearrange("j p c -> p j c"))
        ident_sb = wp.tile([P, P], bf16, tag="ident", name="ident")
        nc.sync.dma_start(ident_sb[:], ident_in.ap())

        # ================= Phase 1: in_proj, silu(z), conv, u, dbl ==========
        with ExitStack() as p1:
            p1w = p1.enter_context(tc.tile_pool(name="p1w", bufs=1))
            x16p = p1.enter_context(tc.tile_pool(name="x16p", bufs=1))
            tp1 = p1.enter_context(tc.tile_pool(name="tp1", bufs=2))

            hT_sb = p1w.tile([P, NKT, L], bf16, tag="hT", name="hT")
            nc.sync.dma_start(hT_sb[:], hT16.ap().rearrange("(k p) l -> p k l", p=P))
            w_in_sb = p1w.tile([P, NKT, 2 * NPAIR * P], bf16, tag="w_in", name="w_in")
            nc.sync.dma_start(w_in_sb[:],
                              w_in16.ap().rearrange("(k p) c -> p k c", p=P))
            convd_sb = p1w.tile([P, 2, NPAIR, KC, P], bf16, tag="convd", name="convd")
            nc.sync.dma_start(convd_sb[:],
                              convd16.ap().rearrange("d j k p m -> p d j k m"))
            w_xp_sb = p1w.tile([P, 2, NPAIR, E], bf16, tag="w_xp", name="w_xp")
            nc.sync.dma_start(w_xp_sb[:],
                              w_xp16.ap().rearrange("d j p e -> p d j e"))

            x16 = [x16p.tile([P, L], bf16, tag=f"x16_{j}", name=f"x16_{j}")
                   for j in range(NPAIR)]

            # ---- in_proj (+ silu(z) -> szd) ----
            def in_proj_half(j, s, sz16=None):
                for c in range(NCH):
                    cols = slice(c * LC, (c + 1) * LC)
                    ps = xzps.tile([P, LC], f32, tag="xzps", name="xzps")
                    wcol = (j * 2 + s) * P
                    for kt in range(NKT):
                        nc.tensor.matmul(
                            ps[:], w_in_sb[:, kt, wcol:wcol + P],
                            hT_sb[:, kt, cols],
                            start=(kt == 0), stop=(kt == NKT - 1))
                    if s == 0:
                        nc.scalar.copy(x16[j][:, cols], ps[:])
                    elif sim_compat:
                        z16 = tp1.tile([P, LC], bf16, tag="z16", name="z16")
                        nc.scalar.copy(z16[:], ps[:])
                        sg = tp1.tile([P, LC], bf16, tag="zsg", name="zsg")
                        nc.scalar.activation(sg[:], ps[:], AF.Sigmoid)
                        nc.vector.tensor_tensor(
                            sz16[:, cols], z16[:], sg[:], op=ALU.mult)
                    else:
                        nc.scalar.activation(sz16[:, cols], ps[:], AF.Silu)

            xzps = p1.enter_context(
                tc.tile_pool(name="xzps", bufs=2, space="PSUM"))
            # x halves first so conv-a/dbl-a/AllReduce-a launch early;
            # z halves later, overlapped with the collective.
            for j in range(NPAIR):
                in_proj_half(j, 0)

            # ---- conv + silu -> u (dir b time-reversed), dbl, collective ----
            for dr in range(2):
                with ExitStack() as s2:
                    up = s2.enter_context(tc.tile_pool(name=f"up{dr}", bufs=1))
                    cps = s2.enter_context(
                        tc.tile_pool(name=f"cps{dr}", bufs=2, space="PSUM"))
                    dblps = s2.enter_context(
                        tc.tile_pool(name=f"dblps{dr}", bufs=2, space="PSUM"))
                    u16 = [up.tile([P, L], bf16, tag=f"u16_{j}", name=f"u16_{dr}{j}")
                           for j in range(NPAIR)]
                    for j in range(NPAIR):
                        urev = u16[j][:, ::-1] if dr == 1 else None
                        for c in range(NCH):
                            c0 = c * LC
                            ps = cps.tile([P, LC], f32, tag="cps", name="cps")
                            if dr == 0:
                                # tap k reads x[t-3+k]; k=3 full width first
                                nc.tensor.matmul(
                                    ps[:], convd_sb[:, dr, j, 3, :],
                                    x16[j][:, c0:c0 + LC],
                                    start=True, stop=False)
                                for k in range(3):
                                    sh = 3 - k
                                    if c == 0:
                                        nc.tensor.matmul(
                                            ps[:, sh:LC],
                                            convd_sb[:, dr, j, k, :],
                                            x16[j][:, 0:LC - sh],
                                            start=False, stop=(k == 2))
                                    else:
                                        nc.tensor.matmul(
                                            ps[:], convd_sb[:, dr, j, k, :],
                                            x16[j][:, c0 - sh:c0 + LC - sh],
                                            start=False, stop=(k == 2))
                            else:
                                # flipped tap jj reads x[t+jj]; jj=0 full first
                                nc.tensor.matmul(
                                    ps[:], convd_sb[:, dr, j, 0, :],
                                    x16[j][:, c0:c0 + LC],
                                    start=True, stop=False)
                                for jj in range(1, KC):
                                    if c == NCH - 1:
                                        nc.tensor.matmul(
                                            ps[:, 0:LC - jj],
                                            convd_sb[:, dr, j, jj, :],
                                            x16[j][:, c0 + jj:L],
                                            start=False, stop=(jj == KC - 1))
                                    else:
                                        nc.tensor.matmul(
                                            ps[:], convd_sb[:, dr, j, jj, :],
                                            x16[j][:, c0 + jj:c0 + LC + jj],
                                            start=False, stop=(jj == KC - 1))
                            if dr == 0:
                                uout = u16[j][:, c0:c0 + LC]
                            else:
                                uout = urev[:, c0:c0 + LC]
                            if sim_compat:
                                pre = tp1.tile([P, LC], bf16, tag="upre", name="upre")
                                nc.scalar.activation(pre[:], ps[:], AF.Identity,
                                                     bias=cb_sb[:, dr, j:j + 1])
                                sgu = tp1.tile([P, LC], bf16, tag="usg", name="usg")
                                nc.scalar.activation(sgu[:], ps[:], AF.Sigmoid,
                                                     bias=cb_sb[:, dr, j:j + 1])
                                nc.vector.tensor_tensor(uout, pre[:], sgu[:],
                                                        op=ALU.mult)
                            else:
                                nc.scalar.activation(uout, ps[:], AF.Silu,
                                                     bias=cb_sb[:, dr, j:j + 1])
                        nc.sync.dma_start(u_dram.ap()[dr, j], u16[j][:])

                    # dbl partials over this core's channels (tau coords for b)
                    dbl_sb = up.tile([E, L], bf16, tag="dbl", name=f"dbl{dr}")
                    for c in range(NCH):
                        cols = slice(c * LC, (c + 1) * LC)
                        dps = dblps.tile([E, LC], f32, tag="dblps", name="dblps")
                        for j in range(NPAIR):
                            # u16 is tau-ordered for dr=1; dbl follows branch coords
                            nc.tensor.matmul(
                                dps[:], w_xp_sb[:, dr, j, :], u16[j][:, cols],
                                start=(j == 0), stop=(j == NPAIR - 1))
                        nc.scalar.copy(dbl_sb[:, cols], dps[:])
                    nc.sync.dma_start(cc_in.ap()[dr], dbl_sb[:])

                nc.gpsimd.collective_compute(
                    "AllReduce", ALU.add,
                    replica_groups=[[0, 1, 2, 3], [4, 5, 6, 7]],
                    ins=[cc_in.ap()[dr]], outs=[cc_out.ap()[dr]])

                if dr == 0:
                    # z halves + silu gate, overlapped with AllReduce-a
                    for j in range(NPAIR):
                        sz16 = tp1.tile([P, L], bf16, tag="sz16", name="sz16")
                        in_proj_half(j, 1, sz16)
                        nc.sync.dma_start(szd.ap()[j], sz16[:])

        # ================= Phase 2: delta, scans, gate =======================
        for dr in range(2):
            with ExitStack() as p2:
                dlp = p2.enter_context(tc.tile_pool(name="dlp", bufs=1))
                bp = p2.enter_context(tc.tile_pool(name="bp", bufs=2))
                trp = p2.enter_context(tc.tile_pool(name="trp", bufs=2))
                hp = p2.enter_context(tc.tile_pool(name="hp", bufs=2))
                dup = p2.enter_context(tc.tile_pool(name="dup", bufs=2))
                dup1 = p2.enter_context(tc.tile_pool(name="dup1", bufs=1))
                gp = p2.enter_context(tc.tile_pool(name="gp", bufs=1))

                # dt_low slice of the bf16 AllReduce output; borrows the gate
                # pool's y16 tag (disjoint lifetime).
                dtlow = gp.tile([DTR, L], bf16, tag="y16", name="dtlow")
                nc.sync.dma_start(dtlow[:], cc_out.ap()[dr, 0:DTR, :])

                # delta per j (kept for the whole dir)
                d16 = [dlp.tile([P, L], bf16, tag=f"d16_{j}", name=f"d16_{dr}{j}")
                       for j in range(NPAIR)]
                with ExitStack() as sdt:
                    psd = sdt.enter_context(
                        tc.tile_pool(name="psd", bufs=2, space="PSUM"))
                    # e16 borrows gate-pool tags (disjoint lifetimes, same shape)
                    e16 = [gp.tile([P, L], bf16, tag=t, name=f"e16{t}")
                           for t in ("u_a", "m_a", "sz")]
                    for j in range(NPAIR):
                        for c in range(NCH):
                            cols = slice(c * LC, (c + 1) * LC)
                            dps = psd.tile([P, LC], f32, tag="dtps", name="dtps")
                            nc.tensor.matmul(dps[:], w_dt_sb[:, dr, j, :],
                                             dtlow[:, cols],
                                             start=True, stop=True)
                            nc.scalar.activation(e16[j][:, cols], dps[:],
                                                 AF.Exp,
                                                 bias=dtb_sb[:, dr, j:j + 1])
                    for j in range(NPAIR):
                        for c in range(NCH):
                            cols = slice(c * LC, (c + 1) * LC)
                            nc.scalar.activation(d16[j][:, cols],
                                                 e16[j][:, cols], AF.Ln,
                                                 bias=1.0)

                mps = p2.enter_context(
                    tc.tile_pool(name="mps", bufs=1, space="PSUM"))
                if dr == 1:
                    dd_a, dd_b = [], []
                    for j in range(NPAIR):
                        da = dlp.tile([P, P], bf16, tag=f"dda{j}", name=f"dda{j}")
                        nc.vector.tensor_scalar_mul(da[:], ident_sb[:],
                                                    D_sb[:, 0, j:j + 1])
                        dd_a.append(da)
                        db = dlp.tile([P, P], bf16, tag=f"ddb{j}", name=f"ddb{j}")
                        nc.vector.tensor_scalar_mul(db[:], ident_sb[:],
                                                    D_sb[:, 1, j:j + 1])
                        dd_b.append(db)
                for j in range(NPAIR):
                    u16j = dup.tile([P, L], bf16, tag="u16j", name="u16j")
                    nc.sync.dma_start(u16j[:], u_dram.ap()[dr, j])
                    du16 = dup1.tile([P, L], bf16, tag="du16", name="du16")
                    for hf in range(2):
                        hc_ = slice(hf * LH, (hf + 1) * LH)
                        nc.vector.tensor_tensor(du16[:, hc_], d16[j][:, hc_],
                                                u16j[:, hc_], op=ALU.mult)

                    m_ps = [mps.tile([P, LC], f32, tag=f"mps{c}", name=f"mps{c}")
                            for c in range(NCH)]
                    for n in range(DS):
                        Brep = bp.tile([P, L], bf16, tag="Brep", name="Brep")
                        nc.sync.dma_start(
                            Brep[:],
                            cc_out.ap()[dr, DTR + n:DTR + n + 1,
                                         :].partition_broadcast(P))
                        Crep = bp.tile([P, L], bf16, tag="Crep", name="Crep")
                        nc.sync.dma_start(
                            Crep[:],
                            cc_out.ap()[dr, DTR + DS + n:DTR + DS + n + 1,
                                         :].partition_broadcast(P))
                        d1 = trp.tile([P, L], bf16, tag="d1", name="d1")
                        for hf in range(2):
                            hc_ = slice(hf * LH, (hf + 1) * LH)
                            nc.vector.tensor_tensor(d1[:, hc_], du16[:, hc_],
                                                    Brep[:, hc_], op=ALU.mult)
                        dA = trp.tile([P, L], bf16, tag="dA", name="dA")
                        nc.scalar.activation(dA[:], d16[j][:], AF.Exp,
                                             scale=Acol_sb[:, dr, j, n:n + 1])
                        h = hp.tile([P, L], bf16, tag="h", name="h")
                        nc.vector.tensor_tensor_scan(
                            h[:], dA[:], d1[:], 0.0,
                            op0=ALU.mult, op1=ALU.add)
                        hcm = trp.tile([P, L], bf16, tag="hcm", name="hcm")
                        for hf in range(2):
                            hc_ = slice(hf * LH, (hf + 1) * LH)
                            nc.vector.tensor_tensor(hcm[:, hc_], h[:, hc_],
                                                    Crep[:, hc_], op=ALU.mult)
                        for c4 in range(NCH):
                            cc = slice(c4 * LC, (c4 + 1) * LC)
                            nc.tensor.matmul(
                                m_ps[c4][:], ident_sb[:], hcm[:, cc],
                                start=(n == 0),
                                stop=(dr == 0 and n == DS - 1))

                    if dr == 0:
                        m16 = gp.tile([P, L], bf16, tag="m16", name="m16")
                        for c in range(NCH):
                            cols = slice(c * LC, (c + 1) * LC)
                            nc.scalar.copy(m16[:, cols], m_ps[c][:])
                        nc.sync.dma_start(m_dram.ap()[j], m16[:])
                    else:
                        # y = (m_a + m_b_rev + Da*u_a + Db*u_b_rev)*sz.
                        # m_a and the D-terms join the tau-ordered PSUM
                        # accumulation as identity/diag matmuls; the single
                        # remaining DVE op per chunk is the sz gate.
                        u_a = gp.tile([P, L], bf16, tag="u_a", name="u_a")
                        nc.sync.dma_start(u_a[:], u_dram.ap()[0, j])
                        m_a = gp.tile([P, L], bf16, tag="m_a", name="m_a")
                        nc.sync.dma_start(m_a[:], m_dram.ap()[j])
                        sz = gp.tile([P, L], bf16, tag="sz", name="sz")
                        nc.sync.dma_start(sz[:], szd.ap()[j])
                        y16 = gp.tile([P, L], bf16, tag="y16", name="y16")
                        marev = m_a[:, ::-1]
                        uarev = u_a[:, ::-1]
                        for ct in range(NCH):
                            cc = slice(ct * LC, (ct + 1) * LC)
                            nc.tensor.matmul(m_ps[ct][:], ident_sb[:],
                                             marev[:, cc],
                                             start=False, stop=False)
                            nc.tensor.matmul(m_ps[ct][:], dd_a[j][:],
                                             uarev[:, cc],
                                             start=False, stop=False)
                            nc.tensor.matmul(m_ps[ct][:], dd_b[j][:],
                                             u16j[:, cc],
                                             start=False, stop=True)
                        for c in range(NCH):
                            cols = slice(c * LC, (c + 1) * LC)
                            nc.vector.tensor_tensor(
                                y16[:, cols], m_ps[NCH - 1 - c][:, ::-1],
                                sz[:, cols], op=ALU.mult)
                        nc.sync.dma_start(y_dram.ap()[j], y16[:])

        # ================= out_proj ==========================================
        with ExitStack() as p3:
            yp = p3.enter_context(tc.tile_pool(name="yp", bufs=2))
            osp = p3.enter_context(tc.tile_pool(name="osp", bufs=2))
            ops_pool = p3.enter_context(
                tc.tile_pool(name="ops", bufs=2, space="PSUM"))
            for c in range(NCH):
                cols = slice(c * LC, (c + 1) * LC)
                ygc = []
                for j in range(NPAIR):
                    yg = yp.tile([P, LC], bf16, tag=f"yg{j}", name=f"yg{j}")
                    nc.sync.dma_start(yg[:], y_dram.ap()[j, :, cols])
                    ygc.append(yg)
                for ot in range(D // P):
                    ops_ = ops_pool.tile([P, LC], f32, tag="outps", name="outps")
                    for j in range(NPAIR):
                        nc.tensor.matmul(
                            ops_[:], w_out_sb[:, j, ot * P:(ot + 1) * P],
                            ygc[j][:], start=(j == 0), stop=(j == NPAIR - 1))
                    osb = osp.tile([P, LC], f32, tag="osb", name="osb")
                    nc.scalar.copy(osb[:], ops_[:])
                    nc.sync.dma_start(
                        out_part.ap()[ot * P:(ot + 1) * P, cols], osb[:])

    nc.compile()
    return nc


def _prep_core_inputs(inputs, core):
    """Host-side slicing/transposition of full inputs for one core."""
    b = core // 4
    dtiles = [(core % 4) * NPAIR + k for k in range(NPAIR)]
    chans = np.concatenate([np.arange(dt * P, (dt + 1) * P) for dt in dtiles])

    hid = np.asarray(inputs['hidden_states'])
    w_in_full = np.asarray(inputs['in_proj_w'])
    w_out_full = np.asarray(inputs['out_proj_w'])

    per_dir = {}
    for d, sfx in enumerate(('a', 'b')):
        per_dir[d] = dict(
            cw=np.asarray(inputs[f'conv_w_{sfx}'])[chans],
            cb=np.asarray(inputs[f'conv_b_{sfx}'])[chans],
            xp=np.asarray(inputs[f'x_proj_{sfx}_w'])[:, chans],
            dtp=np.asarray(inputs[f'dt_proj_{sfx}_w'])[chans],
            dtb=np.asarray(inputs[f'dt_bias_{sfx}'])[chans],
            A=-np.exp(np.asarray(inputs[f'A_{sfx}_log'])[chans]),
            Dv=np.asarray(inputs[f'D_{sfx}'])[chans],
        )

    w_in_cols = np.empty((D, 2 * NPAIR * P), np.float32)
    for j in range(NPAIR):
        ch_j = chans[j * P:(j + 1) * P]
        w_in_cols[:, (2 * j) * P:(2 * j + 1) * P] = w_in_full[ch_j].T
        w_in_cols[:, (2 * j + 1) * P:(2 * j + 2) * P] = w_in_full[DI + ch_j].T

    # conv taps as diagonal matrices (dir b taps host-flipped)
    convd = np.zeros((2, NPAIR, KC, P, P), np.float32)
    ar = np.arange(P)
    for d in range(2):
        cw = per_dir[d]['cw'].reshape(NPAIR, P, KC)
        if d == 1:
            cw = cw[:, :, ::-1]
        for j in range(NPAIR):
            for k in range(KC):
                convd[d, j, k, ar, ar] = cw[j, :, k]

    out = {
        'hT16': hid[b].T.astype(bfdt),
        'w_in16': w_in_cols.astype(bfdt),
        'convd16': convd.astype(bfdt),
        'cbias': np.ascontiguousarray(
            np.stack([per_dir[d]['cb'].reshape(NPAIR, P) for d in range(2)])
        ).astype(np.float32),
        'w_xp16': np.ascontiguousarray(
            np.stack([per_dir[d]['xp'].T.reshape(NPAIR, P, E)
                      for d in range(2)])).astype(bfdt),
        'w_dt16': np.ascontiguousarray(
            np.stack([per_dir[d]['dtp'].reshape(NPAIR, P, DTR)
                      .transpose(0, 2, 1) for d in range(2)])).astype(bfdt),
        'dtb': np.ascontiguousarray(
            np.stack([per_dir[d]['dtb'].reshape(NPAIR, P)
                      for d in range(2)])).astype(np.float32),
        'Acol': np.ascontiguousarray(
            np.stack([per_dir[d]['A'].reshape(NPAIR, P, DS)
                      for d in range(2)])).astype(np.float32),
        'Dvec': np.ascontiguousarray(
            np.stack([per_dir[d]['Dv'].reshape(NPAIR, P)
                      for d in range(2)])).astype(np.float32),
        'w_out16': np.ascontiguousarray(
            w_out_full[:, chans].T.reshape(NPAIR, P, D)).astype(bfdt),
        'ident_in': np.eye(P, dtype=bfdt),
    }
    return {k: np.ascontiguousarray(v) for k, v in out.items()}


_module_cache = {}


def _get_module():
    key = ('sim' if SIM_COMPAT else 'hw')
    if key not in _module_cache:
        _module_cache[key] = build_module()
    return _module_cache[key]


def kernel(**inputs):
    nc = _get_module()
    in_maps = [_prep_core_inputs(inputs, c) for c in range(NCORES)]
    res = run_bass_kernel_spmd(nc, in_maps, list(range(NCORES)))
    out = np.zeros((B, L, D), np.float32)
    for c in range(NCORES):
        out[c // 4] += res.results[c]['out_part'].T
    return out
